# revision 6
# baseline (speedup 1.0000x reference)
"""TopK sparse autoencoder fwd on 8 TRN2 cores — screen+rescore encoder.

vs baseline (3-pass fp16 encoder): 1-pass fp16 screen + exact rescore of the
~32 boundary-rank candidates per row.  Selection-critical values are refined
exactly; everything else keeps screen precision (study2/study3 validated:
rel err 3.6e-4, all must-rescore features sit at key-ranks 56..72).

  screen  p1 = xh @ wh.T (1 fp16 PE pass)            [128b,512f] psum per fb
  scratch fp16(p1) -> DRAM (decoder mask source)      ACT copy
  keys    (bits(p1) & 0xFFFF8000) | global_feat_idx   DVE STT (truncated p1
          with a unique 15-bit index in the low mantissa: total order, ties
          impossible, index recoverable from the value)
  cands   top-8 per 256-chunk by key (DVE max8)       [128,768] per bt
  chain   10x (max8 + match_replace): ranks 1..48 -> aug, ranks 49..80 ->
          slots (covers every feature whose exact rank could cross 64)
  rescore slots: indirect-DMA gather W rows (per-partition row index from
          key low bits) + DVE STT mult with accum_out = exact fp32 dot
  tau*    64th of [ranks 1..48 | 32 exact rescores]   8-round chain on 80
  mask    dense: (fp16scratch >= tau*) * value        (as baseline decoder)
  corr    per slot: (exact side - screen side) * dec row, gathered fp16,
          accumulated into recon after the transpose-back, pre-normalize
"""

import sys

sys.path.insert(0, "/opt/trn_rl_repo")

import numpy as np  # noqa: E402

import concourse.bacc as bacc  # noqa: E402
import concourse.mybir as mybir  # noqa: E402
import concourse.tile as tile  # noqa: E402
from concourse.bass import IndirectOffsetOnAxis  # noqa: E402
from concourse.bass_utils import run_bass_kernel_spmd  # noqa: E402

dt = mybir.dt
Alu = mybir.AluOpType
Act = mybir.ActivationFunctionType

N_CORES = 8
E = 768
EC = E // 128  # 6
NEG_FILL = -1e30
NSLOT = 32
NAUG = 48 + NSLOT  # 80
NRND = 10  # chain rounds: ranks 1..80


def build_kernel(NB=4, NFB=48):
    B_loc = NB * 128
    F = NFB * 512

    nc = bacc.Bacc("TRN2", target_bir_lowering=False, debug=False,
                   num_devices=N_CORES)
    xh_in = nc.dram_tensor("xTh", [E, B_loc], dt.float16, kind="ExternalInput").ap()
    wh_in = nc.dram_tensor("wTh", [E, F], dt.float16, kind="ExternalInput").ap()
    x32_in = nc.dram_tensor("x32", [B_loc, E], dt.float32, kind="ExternalInput").ap()
    w32_in = nc.dram_tensor("w32", [F, E], dt.float32, kind="ExternalInput").ap()
    dec_in = nc.dram_tensor("dec16", [F, E], dt.float16, kind="ExternalInput").ap()
    dec32_in = nc.dram_tensor("dec32", [F, E], dt.float32, kind="ExternalInput").ap()
    biasT_in = nc.dram_tensor("biasT", [128, EC], dt.float32, kind="ExternalInput").ap()
    id16_in = nc.dram_tensor("ident16", [128, 128], dt.float16, kind="ExternalInput").ap()
    id32_in = nc.dram_tensor("ident32", [128, 128], dt.float32, kind="ExternalInput").ap()
    consts_in = nc.dram_tensor("consts", [128, 2], dt.uint32, kind="ExternalInput").ap()
    out_ext = nc.dram_tensor("out", [B_loc, E], dt.float32, kind="ExternalOutput").ap()
    proj_scr = nc.dram_tensor("proj_scr", [B_loc, F], dt.float16).ap()

    wh_v = wh_in.rearrange("(ec p) f -> p ec f", p=128)
    xh_v = xh_in.rearrange("(ec p) b -> p ec b", p=128)
    x32_v = x32_in.rearrange("(bt p) e -> p bt e", p=128)
    dec_v = dec_in.rearrange("(blk t p) e -> blk p t e", p=128, t=4)
    out_v = out_ext.rearrange("(bt p) e -> bt p e", p=128)

    with tile.TileContext(nc) as tc:
        with tc.tile_pool(name="persist", bufs=1) as pp:
            xTh = pp.tile([128, EC, B_loc], dt.float16, tag="xTh")
            nc.sync.dma_start(xTh[:], xh_v)
            x32 = pp.tile([128, NB, E], dt.float32, tag="x32")
            nc.sync.dma_start(x32[:], x32_v)
            id16 = pp.tile([128, 128], dt.float16, tag="id16")
            id32 = pp.tile([128, 128], dt.float32, tag="id32")
            nc.sync.dma_start(id16[:], id16_in)
            nc.sync.dma_start(id32[:], id32_in)
            biasT = pp.tile([128, EC], dt.float32, tag="biasT")
            nc.sync.dma_start(biasT[:], biasT_in)
            cst = pp.tile([128, 2], dt.uint32, tag="cst")
            nc.sync.dma_start(cst[:], consts_in)

            cands = [pp.tile([128, 768], dt.float32, tag=f"cand{bt}",
                             name=f"cand{bt}") for bt in range(NB)]
            augs = [pp.tile([128, NAUG], dt.float32, tag=f"aug{bt}",
                            name=f"aug{bt}") for bt in range(NB)]
            slots = [pp.tile([128, NSLOT], dt.float32, tag=f"slot{bt}",
                             name=f"slot{bt}") for bt in range(NB)]
            exacts = [pp.tile([128, NSLOT], dt.float32, tag=f"ex{bt}",
                              name=f"ex{bt}") for bt in range(NB)]
            gi32s = [pp.tile([128, NSLOT], dt.uint32, tag=f"gi{bt}",
                             name=f"gi{bt}") for bt in range(NB)]
            corrs = [pp.tile([128, NSLOT], dt.float32, tag=f"corr{bt}",
                             name=f"corr{bt}") for bt in range(NB)]
            accs = [pp.tile([128, E], dt.float32, tag=f"acc{bt}",
                            name=f"acc{bt}") for bt in range(NB)]
            NPF = 1
            pf_d16 = [pp.tile([128, 4, E], dt.float16, tag=f"pfd{k}",
                              name=f"pfd{k}") for k in range(NPF)]
            pf_st = [[pp.tile([128, 512], dt.float16, tag=f"pfs{k}_{bt}",
                              name=f"pfs{k}_{bt}") for bt in range(NB)]
                     for k in range(NPF)]
            taustars = []

            def phase2(bt, p2sb):
                """chains + rescore + tau* + corr weights for one bt."""
                # 10 rounds: ranks 1..48 -> aug cols 0:48, 49..80 -> slots
                for r in range(NRND):
                    if r < 6:
                        m8 = augs[bt][:, r * 8:(r + 1) * 8]
                    else:
                        m8 = slots[bt][:, (r - 6) * 8:(r - 5) * 8]
                    nc.vector.max(m8, cands[bt][:])
                    if r < NRND - 1:
                        nc.vector.match_replace(cands[bt][:], m8, cands[bt][:],
                                                NEG_FILL)
                # global feature idx = key & 0x7fff
                zer = p2sb.tile([128, NSLOT], dt.uint32, tag="zer",
                                name=f"zer{bt}")
                nc.vector.memset(zer[:], 0)
                nc.vector.scalar_tensor_tensor(
                    gi32s[bt][:], slots[bt][:].bitcast(dt.uint32),
                    cst[:, 1:2], zer[:], op0=Alu.bitwise_and,
                    op1=Alu.bitwise_or)
                # rescore each slot: gather W row per partition, exact dot
                junk = pp.tile([128, E], dt.float32, tag=f"junk{bt}",
                               name=f"junk{bt}")
                for s in range(NSLOT):
                    wg = p2sb.tile([128, E], dt.float32, tag="wg",
                                   name=f"wg{bt}_{s}")
                    nc.gpsimd.indirect_dma_start(
                        wg[:], None, w32_in,
                        IndirectOffsetOnAxis(ap=gi32s[bt][:, s:s + 1], axis=0))
                    nc.vector.scalar_tensor_tensor(
                        junk[:], wg[:], 1.0, x32[:, bt, :],
                        op0=Alu.mult, op1=Alu.mult,
                        accum_out=exacts[bt][:, s:s + 1])
                nc.vector.tensor_scalar_add(augs[bt][:, 48:48 + NSLOT], exacts[bt][:], 0.0)
                # tau* = 64th of aug
                t8 = None
                for r in range(8):
                    pool_ = pp if r == 7 else p2sb
                    t8 = pool_.tile([128, 8], dt.float32, tag=f"t8_{bt}_{r}",
                                    name=f"t8_{bt}_{r}")
                    nc.vector.max(t8[:], augs[bt][:])
                    if r < 7:
                        nc.vector.match_replace(augs[bt][:], t8[:],
                                                augs[bt][:], NEG_FILL)
                taustars.append(t8)
                # corr = (exact side) - (screen side), fp16-consistent
                s16c = p2sb.tile([128, NSLOT], dt.float16, tag="s16c",
                                 name=f"s16c_{bt}")
                nc.scalar.copy(s16c[:], slots[bt][:])
                ws = p2sb.tile([128, NSLOT], dt.float32, tag="ws",
                               name=f"ws{bt}")
                nc.vector.scalar_tensor_tensor(ws[:], s16c[:], t8[:, 7:8],
                                               s16c[:], op0=Alu.is_ge,
                                               op1=Alu.mult)
                wex = p2sb.tile([128, NSLOT], dt.float32, tag="wex",
                                name=f"wex{bt}")
                nc.vector.scalar_tensor_tensor(wex[:], exacts[bt][:],
                                               t8[:, 7:8], exacts[bt][:],
                                               op0=Alu.is_ge, op1=Alu.mult)
                nc.vector.tensor_tensor(corrs[bt][:], wex[:], ws[:],
                                        op=Alu.subtract)
                nc.vector.memset(accs[bt][:], 0.0)

            # ---------------- Phase 1: screen encoder + keys + cands ----------------
            with nc.named_scope("phase1"), \
                 tc.tile_pool(name="p1w", bufs=4) as p1w, \
                 tc.tile_pool(name="p1sb", bufs=5) as p1sb, \
                 tc.tile_pool(name="p1io", bufs=6) as p1io, \
                 tc.tile_pool(name="p2sb", bufs=14) as p2sb, \
                 tc.tile_pool(name="p1eps", bufs=4, space="PSUM") as p1eps:

                def w_load(fb):
                    wTh = p1w.tile([128, EC, 512], dt.float16, tag="wTh",
                                   name=f"wTh{fb}")
                    nc.sync.dma_start(wTh[:], wh_v[:, :, fb * 512:(fb + 1) * 512])
                    iot = p1io.tile([128, 512], dt.uint32, tag="iot",
                                    name=f"iot{fb}")
                    nc.gpsimd.iota(iot[:], pattern=[[1, 512]], base=fb * 512,
                                   channel_multiplier=0)
                    return wTh, iot

                def encode(fb, bt, wpair):
                    wTh, iot = wpair
                    eps = p1eps.tile([128, 512], dt.float32, tag="encps",
                                     name=f"encps{fb}_{bt}")
                    for ec in range(EC):
                        nc.tensor.matmul(
                            eps[:],
                            xTh[:, ec, bt * 128:(bt + 1) * 128],
                            wTh[:, ec, :],
                            start=(ec == 0), stop=(ec == EC - 1))
                    ptile = p1sb.tile([128, 512], dt.float32, tag="ptile",
                                      name=f"ptile{fb}_{bt}")
                    nc.scalar.copy(ptile[:], eps[:])
                    key = p1sb.tile([128, 512], dt.uint32, tag="key",
                                    name=f"key{fb}_{bt}")
                    nc.vector.scalar_tensor_tensor(
                        key[:], ptile[:].bitcast(dt.uint32),
                        cst[:, 0:1], iot[:], op0=Alu.bitwise_and,
                        op1=Alu.bitwise_or)
                    keyf = key[:].bitcast(dt.float32)
                    st16 = p1sb.tile([128, 512], dt.float16, tag="st16",
                                     name=f"st16_{fb}_{bt}")
                    nc.scalar.copy(st16[:], keyf)
                    nc.sync.dma_start(
                        proj_scr[bt * 128:(bt + 1) * 128, fb * 512:(fb + 1) * 512],
                        st16[:])
                    for seg in range(2):
                        off = fb * 16 + seg * 8
                        nc.vector.max(cands[bt][:, off:off + 8],
                                      keyf[:, seg * 256:(seg + 1) * 256])

                STAG = 4
                NMAIN = NFB - STAG
                wp = {0: w_load(0), 1: w_load(1)}
                for fb in range(NMAIN):
                    if fb + 2 < NFB:
                        wp[fb + 2] = w_load(fb + 2)
                    for bt in range(NB):
                        encode(fb, bt, wp[fb])
                    del wp[fb]
                for fb in range(NMAIN + 2, NFB):
                    wp[fb] = w_load(fb)
                for k in range(NPF):
                    nc.sync.dma_start(pf_d16[k][:], dec_v[k])
                    for bt in range(NB):
                        nc.sync.dma_start(
                            pf_st[k][bt][:],
                            proj_scr[bt * 128:(bt + 1) * 128,
                                     k * 512:(k + 1) * 512])
                # bt-major tail: bt's phase2 overlaps bt+1's encodes
                for bt in range(NB):
                    for fb in range(NMAIN, NFB):
                        encode(fb, bt, wp[fb])
                    phase2(bt, p2sb)

            # ---------------- Phase 3: transposed masked decoder ----------------
            with nc.named_scope("phase3"), \
                 tc.tile_pool(name="p3d16", bufs=7) as p3d16, \
                 tc.tile_pool(name="p3sb", bufs=12) as p3sb, \
                 tc.tile_pool(name="p3mt", bufs=6) as p3mt, \
                 tc.tile_pool(name="p3rps", bufs=1, space="PSUM") as p3rps, \
                 tc.tile_pool(name="p3tps", bufs=2, space="PSUM") as p3tps:
                rps = [p3rps.tile([128, B_loc], dt.float32, tag=f"rps{ec}",
                                  name=f"rps{ec}") for ec in range(EC)]
                for fb in range(NFB):
                    if fb < NPF:
                        d16 = pf_d16[fb]
                    else:
                        d16 = p3d16.tile([128, 4, E], dt.float16, tag="d16",
                                         name=f"d16_{fb}")
                        nc.sync.dma_start(d16[:], dec_v[fb])
                    m16s = []
                    for bt in range(NB):
                        if fb < NPF:
                            stile = pf_st[fb][bt]
                        else:
                            stile = p3sb.tile([128, 512], dt.float16,
                                              tag="stile",
                                              name=f"stile{fb}_{bt}")
                            nc.sync.dma_start(
                                stile[:],
                                proj_scr[bt * 128:(bt + 1) * 128,
                                         fb * 512:(fb + 1) * 512])
                        m16 = p3sb.tile([128, 512], dt.float16, tag="m16",
                                        name=f"m16_{fb}_{bt}")
                        nc.vector.scalar_tensor_tensor(
                            m16[:], stile[:], taustars[bt][:, 7:8], stile[:],
                            op0=Alu.is_ge, op1=Alu.mult)
                        m16s.append(m16)
                    pend = []

                    def flush_one(fb_):
                        fs_, mT_ = pend.pop(0)
                        for ec in range(EC):
                            nc.tensor.matmul(
                                rps[ec][:],
                                d16[:, fs_, ec * 128:(ec + 1) * 128],
                                mT_[:],
                                start=(fb_ == 0 and fs_ == 0),
                                stop=(fb_ == NFB - 1 and fs_ == 3))

                    for fs in range(4):
                        tps = p3tps.tile([128, B_loc], dt.float16, tag="tps",
                                         name=f"tps{fb}_{fs}")
                        for bt in range(NB):
                            nc.tensor.transpose(
                                tps[:, bt * 128:(bt + 1) * 128],
                                m16s[bt][:, fs * 128:(fs + 1) * 128],
                                id16[:])
                        mT = p3mt.tile([128, B_loc], dt.float16, tag="mT",
                                       name=f"mT{fb}_{fs}")
                        nc.scalar.copy(mT[:], tps[:])
                        pend.append((fs, mT))
                        if len(pend) > 2:
                            flush_one(fb)
                    while pend:
                        flush_one(fb)

                for bt in range(NB):
                    for s_ in range(NSLOT):
                        dg = p3sb.tile([128, E], dt.float16, tag="dg",
                                       name=f"dg{bt}_{s_}")
                        nc.gpsimd.indirect_dma_start(
                            dg[:], None, dec_in,
                            IndirectOffsetOnAxis(ap=gi32s[bt][:, s_:s_ + 1],
                                                 axis=0))
                        nc.vector.scalar_tensor_tensor(
                            accs[bt][:], dg[:], corrs[bt][:, s_:s_ + 1],
                            accs[bt][:], op0=Alu.mult, op1=Alu.add)

                rbT = []
                for ec in range(EC):
                    rb = pp.tile([128, B_loc], dt.float32, tag=f"rbT{ec}",
                                 name=f"rbT{ec}")
                    nc.vector.tensor_scalar_add(rb[:], rps[ec][:],
                                                biasT[:, ec:ec + 1])
                    rbT.append(rb)

            # -------- finalize: transpose back, corr decode, normalize --------
            with nc.named_scope("phase4"), \
                 tc.tile_pool(name="p4sb", bufs=2) as p4, \
                 tc.tile_pool(name="p4ps", bufs=2, space="PSUM") as p4ps:
                for bt in range(NB):
                    ops_ = [p4ps.tile([128, 384], dt.float32, tag=f"ops{h}",
                                      name=f"ops{bt}_{h}") for h in range(2)]
                    for ec in range(EC):
                        nc.tensor.transpose(
                            ops_[ec // 3][:, (ec % 3) * 128:(ec % 3 + 1) * 128],
                            rbT[ec][:, bt * 128:(bt + 1) * 128],
                            id32[:])
                    rb = p4.tile([128, E], dt.float32, tag="rb", name=f"rb{bt}")
                    for h in range(2):
                        nc.scalar.copy(rb[:, h * 384:(h + 1) * 384], ops_[h][:])
                    # corr decode: acc16 += corr_s * dec16[f_s] per slot
                    nc.vector.tensor_tensor(rb[:], rb[:], accs[bt][:],
                                            op=Alu.add)
                    sq = p4.tile([128, E], dt.float32, tag="sq", name=f"sq{bt}")
                    nc.vector.tensor_tensor(sq[:], rb[:], rb[:], op=Alu.mult)
                    ss = p4.tile([128, 1], dt.float32, tag="ss", name=f"ss{bt}")
                    nc.vector.tensor_reduce(ss[:], sq[:], axis=mybir.AxisListType.X,
                                            op=Alu.add)
                    nrm = p4.tile([128, 1], dt.float32, tag="nrm", name=f"nrm{bt}")
                    nc.scalar.activation(nrm[:], ss[:], Act.Sqrt)
                    nc.vector.tensor_scalar_max(nrm[:], nrm[:], 1e-12)
                    inv = p4.tile([128, 1], dt.float32, tag="inv", name=f"inv{bt}")
                    nc.vector.reciprocal(inv[:], nrm[:])
                    ot = p4.tile([128, E], dt.float32, tag="ot", name=f"ot{bt}")
                    nc.vector.tensor_scalar_mul(ot[:], rb[:], inv[:])
                    nc.sync.dma_start(out_v[bt], ot[:])

    nc.finalize()
    return nc


_CACHE = {}


def _get_nc(NB, NFB):
    key = (NB, NFB)
    if key not in _CACHE:
        _CACHE[key] = build_kernel(NB, NFB)
    return _CACHE[key]


def _prep_host(embed, enc_bias, enc_weight, dec_lookup, NB):
    B_loc = NB * 128
    xc = (embed - enc_bias[None, :]).astype(np.float32)
    xT = np.ascontiguousarray(xc.T)
    xTh = xT.astype(np.float16)
    wT = np.ascontiguousarray(enc_weight.T)
    wTh = wT.astype(np.float16)
    dec16 = dec_lookup.astype(np.float16)
    biasT = np.ascontiguousarray(enc_bias.reshape(EC, 128).T)
    eye16 = np.eye(128, dtype=np.float16)
    eye32 = np.eye(128, dtype=np.float32)
    consts = np.tile(np.array([[0xFFFF8000, 0x7FFF]], np.uint32), (128, 1))
    w32 = np.ascontiguousarray(enc_weight.astype(np.float32))
    in_maps = []
    for c in range(N_CORES):
        sl = slice(c * B_loc, (c + 1) * B_loc)
        in_maps.append({
            "xTh": np.ascontiguousarray(xTh[:, sl]),
            "x32": np.ascontiguousarray(xc[sl]),
            "wTh": wTh,
            "w32": w32,
            "dec16": dec16,
            "dec32": np.ascontiguousarray(dec_lookup.astype(np.float32)),
            "biasT": biasT,
            "ident16": eye16,
            "ident32": eye32,
            "consts": consts,
        })
    return in_maps


def run(embed, enc_bias, enc_weight, dec_lookup, NB=4, NFB=48, trace=False):
    in_maps = _prep_host(embed, enc_bias, enc_weight, dec_lookup, NB)
    nc = _get_nc(NB, NFB)
    res = run_bass_kernel_spmd(nc, in_maps, list(range(N_CORES)), trace=trace)
    out = np.concatenate([res.results[c]["out"] for c in range(N_CORES)], axis=0)
    return out, res


def kernel(embed, enc_bias, enc_weight, dec_lookup):
    import time

    args = (np.asarray(embed, dtype=np.float32),
            np.asarray(enc_bias, dtype=np.float32),
            np.asarray(enc_weight, dtype=np.float32),
            np.asarray(dec_lookup, dtype=np.float32))
    last_exc = None
    for attempt in range(3):
        try:
            out, _ = run(*args)
            return out
        except Exception as e:  # noqa: BLE001
            last_exc = e
            time.sleep(10.0)
    raise last_exc
